# revision 15
# baseline (speedup 1.0000x reference)
"""Trainium2 Bass kernel for nn_CellLineMLPPredictor.

Computation (B=512 samples):
  x0 = concat(h_drug[pairs[:,0]], attrs[:,1:2], h_drug[pairs[:,1]], attrs[:,3:4])  [B, 2048]
  x1 = relu(x0 @ W0.T + b0)      [B, 2048]
  x2 = relu(x1 @ W1.T + b1)      [B, 1024]
  z  = relu(einsum('boi,bi->bo', L0[cl], x2) + O0[cl,:,0])  [B, 512]
  y  = einsum('boi,bi->bo', L1[cl], z) + O1[cl,:,0]          [B, 1] -> [B]

Strategy (8 cores, no collectives):
  - Host routing: samples sorted by cell line. Core c owns cell lines
    [4c, 4c+4); its samples are packed into 4 groups of G padded columns
    (G = max group count rounded to 8). All per-sample gathers (h_drug,
    L1, O0, O1 selection) become dense per-group matmuls.
  - All activations are kept feature-major ("transposed": [features,
    samples]), so every layer is out.T = W @ x.T and the natural [out,
    in] weight layout transposed once on host gives lhsT tiles directly.
  - W0/W1 replicated per core and streamed in fp16; L0 is expert-sharded
    so it is read exactly once across the machine. PSUM accumulation and
    bias+relu epilogues in fp32.
  - Weights are host-packed into [chunk, 128, 4096] so each DMA is one
    fully-contiguous ~1MB transfer (the Sync sequencer's per-DMA trigger
    cost is ~0.6us, so few big DMAs beat many small ones). Small consts
    and the output go through the Scalar engine's HWDGE ring instead.
"""

import numpy as np


try:
    import concourse.bass  # noqa: F401
except ImportError:  # grading environment may not have it on sys.path
    import sys

    for _p in ("/opt/trn_rl_repo", "/root/.axon_site/_ro/trn_rl_repo"):
        if _p not in sys.path:
            sys.path.insert(0, _p)

B = 512
N_CELL = 32
N_CORE = 8
GROUPS_PER_CORE = N_CELL // N_CORE  # 4
D_IN = 2048
P = 128  # partitions

LAST_RUN = None  # BassKernelResults of the most recent kernel() call
_PROG_CACHE = {}  # G -> compiled Bass program (avoids recompiling on repeat calls)


def _get_program(G):
    if G not in _PROG_CACHE:
        _PROG_CACHE[G] = _build_program(G)
    return _PROG_CACHE[G]


def _build_program(G):
    """Build the SPMD Bass program. G = padded per-group column count."""
    import concourse.bacc as bacc
    import concourse.mybir as mybir
    from concourse.tile import TileContext

    f32 = mybir.dt.float32
    f16 = mybir.dt.float16
    Relu = mybir.ActivationFunctionType.Relu
    Identity = mybir.ActivationFunctionType.Identity
    Add = mybir.AluOpType.add
    Max = mybir.AluOpType.max

    NCOL = GROUPS_PER_CORE * G  # columns (samples) per core

    nc = bacc.Bacc("TRN2", target_bir_lowering=False)

    # Per-core inputs (pre-packed on host into SBUF-ready layouts).
    # Weight packs are [n_chunks, 128, 4096]: each chunk is 4 contraction
    # tiles side by side in the free dim, one contiguous 1MB DMA.
    x0p = nc.dram_tensor("x0p", [P, 16 * NCOL], f16, kind="ExternalInput")
    w0p = nc.dram_tensor("w0p", [8, P, 4096], f16, kind="ExternalInput")
    w1p = nc.dram_tensor("w1p", [4, P, 4096], f16, kind="ExternalInput")
    l0p = nc.dram_tensor("l0p", [4, P, 4096], f16, kind="ExternalInput")
    b0m = nc.dram_tensor("b0m", [P, 16], f32, kind="ExternalInput")
    b1m = nc.dram_tensor("b1m", [P, 8], f32, kind="ExternalInput")
    o0m = nc.dram_tensor("o0m", [P, 16], f32, kind="ExternalInput")
    l1m = nc.dram_tensor("l1m", [P, 16], f16, kind="ExternalInput")
    o1m = nc.dram_tensor("o1m", [1, 4], f32, kind="ExternalInput")
    y = nc.dram_tensor("y", [1, NCOL], f32, kind="ExternalOutput")

    with TileContext(nc) as tc:
        with (
            tc.tile_pool(name="consts", bufs=1) as consts,
            tc.tile_pool(name="acts", bufs=1) as acts,
            tc.tile_pool(name="wpool", bufs=12) as wpool,
            tc.tile_pool(name="l0pool", bufs=4) as l0pool,
            tc.tile_pool(name="psum", bufs=8, space="PSUM") as psum,
        ):
            x0sb = acts.tile([P, 16 * NCOL], f16)
            nc.sync.dma_start(x0sb[:], x0p[:])
            # consts go through the Scalar HWDGE ring to keep the Sync
            # ring free for the weight stream
            b0sb = consts.tile([P, 16], f32, tag="b0sb")
            nc.scalar.dma_start(b0sb[:], b0m[:])
            b1sb = consts.tile([P, 8], f32, tag="b1sb")
            nc.scalar.dma_start(b1sb[:], b1m[:])
            o0sb = consts.tile([P, 16], f32, tag="o0sb")
            nc.scalar.dma_start(o0sb[:], o0m[:])
            l1sb = consts.tile([P, 16], f16, tag="l1sb")
            nc.scalar.dma_start(l1sb[:], l1m[:])
            o1sb = consts.tile([1, 4], f32, tag="o1sb")
            nc.scalar.dma_start(o1sb[:], o1m[:])

            x1sb = acts.tile([P, 16 * NCOL], f16, tag="x1sb")
            x2sb = acts.tile([P, 8 * NCOL], f16, tag="x2sb")
            zsb = acts.tile([P, 16 * G], f16, tag="zsb")
            ysb = acts.tile([1, NCOL], f32, tag="ysb")

            # ---- stage 1: x1.T = relu(W0 @ x0.T + b0), M=2048 in 2 halves
            for mh in range(2):
                ps = [
                    psum.tile([P, NCOL], f32, tag="ps", name=f"ps{i}")
                    for i in range(8)
                ]
                for c in range(4):
                    wt = wpool.tile([P, 4096], f16, tag="w", name="wt")
                    nc.sync.dma_start(wt[:, :2048], w0p[mh * 4 + c][:, :2048])
                    nc.scalar.dma_start(wt[:, 2048:], w0p[mh * 4 + c][:, 2048:])
                    for kk in range(4):
                        k = c * 4 + kk
                        for mi in range(8):
                            nc.tensor.matmul(
                                ps[mi][:],
                                wt[:, kk * 1024 + mi * 128 : kk * 1024 + (mi + 1) * 128],
                                x0sb[:, k * NCOL : (k + 1) * NCOL],
                                start=(k == 0),
                                stop=(k == 15),
                            )
                for mi in range(8):
                    m = mh * 8 + mi
                    nc.scalar.activation(
                        x1sb[:, m * NCOL : (m + 1) * NCOL],
                        ps[mi][:],
                        Relu,
                        bias=b0sb[:, m : m + 1],
                    )

            # ---- stage 2: x2.T = relu(W1 @ x1.T + b1), M=1024
            ps2 = [
                psum.tile([P, NCOL], f32, tag="ps", name=f"ps{i}") for i in range(8)
            ]
            for c in range(4):
                wt = wpool.tile([P, 4096], f16, tag="w", name="wt")
                nc.sync.dma_start(wt[:, :2048], w1p[c][:, :2048])
                nc.scalar.dma_start(wt[:, 2048:], w1p[c][:, 2048:])
                for kk in range(4):
                    k = c * 4 + kk
                    for mi in range(8):
                        nc.tensor.matmul(
                            ps2[mi][:],
                            wt[:, kk * 1024 + mi * 128 : kk * 1024 + (mi + 1) * 128],
                            x1sb[:, k * NCOL : (k + 1) * NCOL],
                            start=(k == 0),
                            stop=(k == 15),
                        )
            for mi in range(8):
                nc.scalar.activation(
                    x2sb[:, mi * NCOL : (mi + 1) * NCOL],
                    ps2[mi][:],
                    Relu,
                    bias=b1sb[:, mi : mi + 1],
                )

            # ---- stage 3: per group g: z_g.T = relu(L0[c_g] @ x2_g.T + O0)
            # l0p[g] holds L0[c_g].T as 8 k-tiles of [128, 512] side by side
            lts = []
            for h in range(4):
                lt = l0pool.tile([P, 4096], f16, tag="l0", name=f"lt{h}")
                nc.sync.dma_start(lt[:, :2048], l0p[h][:, :2048])
                nc.scalar.dma_start(lt[:, 2048:], l0p[h][:, 2048:])
                lts.append(lt)
            for g in range(GROUPS_PER_CORE):
                ps3 = [
                    psum.tile([P, G], f32, tag="ps", name=f"ps3_{i}")
                    for i in range(4)
                ]
                wt = lts[g]
                base = 0
                for k in range(8):
                    for mi in range(4):
                        nc.tensor.matmul(
                            ps3[mi][:],
                            wt[:, base + k * 512 + mi * 128 : base + k * 512 + (mi + 1) * 128],
                            x2sb[:, k * NCOL + g * G : k * NCOL + (g + 1) * G],
                            start=(k == 0),
                            stop=(k == 7),
                        )
                for mi in range(4):
                    # fused (psum + O0) then max(0) on the otherwise-idle DVE
                    nc.vector.tensor_scalar(
                        zsb[:, (g * 4 + mi) * G : (g * 4 + mi + 1) * G],
                        ps3[mi][:],
                        o0sb[:, g * 4 + mi : g * 4 + mi + 1],
                        0.0,
                        Add,
                        Max,
                    )

            # ---- stage 4: y_g = L1[c_g] @ z_g.T + O1  -> [1, G] per group
            for g in range(GROUPS_PER_CORE):
                ps4 = psum.tile([1, G], f32, tag="ps", name="ps4")
                for k in range(4):
                    nc.tensor.matmul(
                        ps4[:],
                        l1sb[:, g * 4 + k : g * 4 + k + 1],
                        zsb[:, (g * 4 + k) * G : (g * 4 + k + 1) * G],
                        start=(k == 0),
                        stop=(k == 3),
                    )
                nc.scalar.activation(
                    ysb[0:1, g * G : (g + 1) * G],
                    ps4[0:1, :],
                    Identity,
                    bias=o1sb[0:1, g : g + 1],
                )

            nc.scalar.dma_start(y[:], ysb[:])

    nc.compile()
    return nc


def kernel(**inputs):
    global LAST_RUN
    import os

    from concourse.bass_utils import run_bass_kernel_spmd

    pairs = np.asarray(inputs["pairs"]).astype(np.int64)
    cell_lines = np.asarray(inputs["cell_lines"]).astype(np.int64)
    attrs = np.asarray(inputs["attrs"], dtype=np.float32)
    h_drug = np.asarray(inputs["h_drug"], dtype=np.float32)
    W0 = np.asarray(inputs["W0"], dtype=np.float32)
    b0 = np.asarray(inputs["b0"], dtype=np.float32)
    W1 = np.asarray(inputs["W1"], dtype=np.float32)
    b1 = np.asarray(inputs["b1"], dtype=np.float32)
    L0 = np.asarray(inputs["L0"], dtype=np.float32)
    O0 = np.asarray(inputs["O0"], dtype=np.float32)
    L1 = np.asarray(inputs["L1"], dtype=np.float32)
    O1 = np.asarray(inputs["O1"], dtype=np.float32)

    n_attr = attrs.shape[1] // 2
    # x0.T, feature-major: [2048, B]
    x0T = np.empty((D_IN, B), dtype=np.float32)
    x0T[:1023] = h_drug[pairs[:, 0]].T
    x0T[1023] = attrs[:, n_attr - 1]
    x0T[1024:2047] = h_drug[pairs[:, 1]].T
    x0T[2047] = attrs[:, -1]

    counts = np.bincount(cell_lines, minlength=N_CELL)
    G = max(8, int(-(-counts.max() // 8) * 8))
    NCOL = GROUPS_PER_CORE * G
    # one PSUM bank per [128, NCOL] f32 accumulator; 8 live at once
    assert NCOL <= 512, f"group padding {G} too large for single-bank PSUM tiles"
    groups = [np.where(cell_lines == c)[0] for c in range(N_CELL)]

    # shared (replicated) weight packs, bf16, chunk-of-4-ktiles layout
    w0k = W0.reshape(2, 1024, 16, P).transpose(0, 2, 3, 1).reshape(32, P, 1024)
    w0p = np.ascontiguousarray(
        w0k.reshape(2, 4, 4, P, 1024).transpose(0, 1, 3, 2, 4).reshape(8, P, 4096)
    ).astype(np.float16)
    w1k = W1.T.reshape(16, P, 1024)
    w1p = np.ascontiguousarray(
        w1k.reshape(4, 4, P, 1024).transpose(0, 2, 1, 3).reshape(4, P, 4096)
    ).astype(np.float16)
    b0m = np.ascontiguousarray(b0.reshape(16, P).T)
    b1m = np.ascontiguousarray(b1.reshape(8, P).T)

    in_maps = []
    for core in range(N_CORE):
        cells = [GROUPS_PER_CORE * core + i for i in range(GROUPS_PER_CORE)]
        x0c = np.zeros((D_IN, NCOL), dtype=np.float32)
        for gi, c in enumerate(cells):
            idx = groups[c]
            x0c[:, gi * G : gi * G + len(idx)] = x0T[:, idx]
        x0p = np.ascontiguousarray(
            x0c.reshape(16, P, NCOL).transpose(1, 0, 2).reshape(P, 16 * NCOL)
        ).astype(np.float16)
        # l0p[g] = L0[c_g].T as [8 ktiles, 128, 512] -> [128, 8*512]
        l0p = np.ascontiguousarray(
            np.stack(
                [
                    L0[c].T.reshape(8, P, 512).transpose(1, 0, 2).reshape(P, 4096)
                    for c in cells
                ]
            )
        ).astype(np.float16)
        o0m = np.ascontiguousarray(
            np.stack([O0[c][:, 0].reshape(4, P) for c in cells])
            .transpose(2, 0, 1)
            .reshape(P, 16)
        )
        l1m = np.ascontiguousarray(
            np.stack([L1[c][0].reshape(4, P) for c in cells])
            .transpose(2, 0, 1)
            .reshape(P, 16)
        ).astype(np.float16)
        o1m = np.ascontiguousarray(
            np.array([[O1[c, 0, 0] for c in cells]], dtype=np.float32)
        )
        in_maps.append(
            {
                "x0p": x0p,
                "w0p": w0p,
                "w1p": w1p,
                "l0p": l0p,
                "b0m": b0m,
                "b1m": b1m,
                "o0m": o0m,
                "l1m": l1m,
                "o1m": o1m,
            }
        )

    nc = _get_program(G)
    trace = bool(os.environ.get("BENCH_TRACE"))
    LAST_RUN = run_bass_kernel_spmd(nc, in_maps, list(range(N_CORE)), trace=trace)
    results = LAST_RUN.results

    out = np.zeros(B, dtype=np.float32)
    for core in range(N_CORE):
        yc = results[core]["y"]
        for gi in range(GROUPS_PER_CORE):
            c = GROUPS_PER_CORE * core + gi
            idx = groups[c]
            out[idx] = yc[0, gi * G : gi * G + len(idx)]
    return out


# revision 16
# speedup vs baseline: 1.0122x; 1.0122x over previous
"""Trainium2 Bass kernel for nn_CellLineMLPPredictor.

Computation (B=512 samples):
  x0 = concat(h_drug[pairs[:,0]], attrs[:,1:2], h_drug[pairs[:,1]], attrs[:,3:4])  [B, 2048]
  x1 = relu(x0 @ W0.T + b0)      [B, 2048]
  x2 = relu(x1 @ W1.T + b1)      [B, 1024]
  z  = relu(einsum('boi,bi->bo', L0[cl], x2) + O0[cl,:,0])  [B, 512]
  y  = einsum('boi,bi->bo', L1[cl], z) + O1[cl,:,0]          [B, 1] -> [B]

Strategy (8 cores, no collectives):
  - Host routing: samples sorted by cell line. Core c owns cell lines
    [4c, 4c+4); its samples are packed into 4 groups of G padded columns
    (G = max group count rounded to 8). All per-sample gathers (h_drug,
    L1, O0, O1 selection) become dense per-group matmuls.
  - All activations are kept feature-major ("transposed": [features,
    samples]), so every layer is out.T = W @ x.T and the natural [out,
    in] weight layout transposed once on host gives lhsT tiles directly.
  - W0/W1 replicated per core and streamed in fp16; L0 is expert-sharded
    so it is read exactly once across the machine. PSUM accumulation and
    bias+relu epilogues in fp32.
  - Weights are host-packed into [chunk, 128, 4096] so each DMA is one
    fully-contiguous ~1MB transfer (the Sync sequencer's per-DMA trigger
    cost is ~0.6us, so few big DMAs beat many small ones). Small consts
    and the output go through the Scalar engine's HWDGE ring instead.
"""

import numpy as np


try:
    import concourse.bass  # noqa: F401
except ImportError:  # grading environment may not have it on sys.path
    import sys

    for _p in ("/opt/trn_rl_repo", "/root/.axon_site/_ro/trn_rl_repo"):
        if _p not in sys.path:
            sys.path.insert(0, _p)

B = 512
N_CELL = 32
N_CORE = 8
GROUPS_PER_CORE = N_CELL // N_CORE  # 4
D_IN = 2048
P = 128  # partitions

LAST_RUN = None  # BassKernelResults of the most recent kernel() call
_PROG_CACHE = {}  # G -> compiled Bass program (avoids recompiling on repeat calls)


def _get_program(G):
    if G not in _PROG_CACHE:
        _PROG_CACHE[G] = _build_program(G)
    return _PROG_CACHE[G]


def _build_program(G):
    """Build the SPMD Bass program. G = padded per-group column count."""
    import concourse.bacc as bacc
    import concourse.mybir as mybir
    from concourse.tile import TileContext

    f32 = mybir.dt.float32
    f16 = mybir.dt.float16
    Relu = mybir.ActivationFunctionType.Relu
    Identity = mybir.ActivationFunctionType.Identity
    Add = mybir.AluOpType.add
    Max = mybir.AluOpType.max

    NCOL = GROUPS_PER_CORE * G  # columns (samples) per core

    nc = bacc.Bacc("TRN2", target_bir_lowering=False)

    # Per-core inputs (pre-packed on host into SBUF-ready layouts).
    # Weight packs are [n_chunks, 128, 4096]: each chunk is 4 contraction
    # tiles side by side in the free dim, one contiguous 1MB DMA.
    x0p = nc.dram_tensor("x0p", [P, 16 * NCOL], f16, kind="ExternalInput")
    w0p = nc.dram_tensor("w0p", [8, P, 4096], f16, kind="ExternalInput")
    w1p = nc.dram_tensor("w1p", [4, P, 4096], f16, kind="ExternalInput")
    l0p = nc.dram_tensor("l0p", [4, P, 4096], f16, kind="ExternalInput")
    b0m = nc.dram_tensor("b0m", [P, 16], f32, kind="ExternalInput")
    b1m = nc.dram_tensor("b1m", [P, 8], f32, kind="ExternalInput")
    o0m = nc.dram_tensor("o0m", [P, 16], f32, kind="ExternalInput")
    l1m = nc.dram_tensor("l1m", [P, 16], f16, kind="ExternalInput")
    o1m = nc.dram_tensor("o1m", [1, 4], f32, kind="ExternalInput")
    y = nc.dram_tensor("y", [1, NCOL], f32, kind="ExternalOutput")

    with TileContext(nc) as tc:
        with (
            tc.tile_pool(name="consts", bufs=1) as consts,
            tc.tile_pool(name="acts", bufs=1) as acts,
            tc.tile_pool(name="wpool", bufs=12) as wpool,
            tc.tile_pool(name="l0pool", bufs=4) as l0pool,
            tc.tile_pool(name="psum", bufs=8, space="PSUM") as psum,
        ):
            x0sb = acts.tile([P, 16 * NCOL], f16)
            nc.sync.dma_start(x0sb[:], x0p[:])
            # consts go through the Scalar HWDGE ring to keep the Sync
            # ring free for the weight stream
            b0sb = consts.tile([P, 16], f32, tag="b0sb")
            nc.scalar.dma_start(b0sb[:], b0m[:])
            b1sb = consts.tile([P, 8], f32, tag="b1sb")
            nc.scalar.dma_start(b1sb[:], b1m[:])
            o0sb = consts.tile([P, 16], f32, tag="o0sb")
            nc.scalar.dma_start(o0sb[:], o0m[:])
            l1sb = consts.tile([P, 16], f16, tag="l1sb")
            nc.scalar.dma_start(l1sb[:], l1m[:])
            o1sb = consts.tile([1, 4], f32, tag="o1sb")
            nc.scalar.dma_start(o1sb[:], o1m[:])

            x1sb = acts.tile([P, 16 * NCOL], f16, tag="x1sb")
            x2sb = acts.tile([P, 8 * NCOL], f16, tag="x2sb")
            zsb = acts.tile([P, 16 * G], f16, tag="zsb")
            ysb = acts.tile([1, NCOL], f32, tag="ysb")

            # ---- stage 1: x1.T = relu(W0 @ x0.T + b0), M=2048 in 2 halves
            for mh in range(2):
                ps = [
                    psum.tile([P, NCOL], f32, tag="ps", name=f"ps{i}")
                    for i in range(8)
                ]
                for c in range(4):
                    wt = wpool.tile([P, 4096], f16, tag="w", name="wt")
                    eng = nc.sync if c % 2 == 0 else nc.scalar
                    eng.dma_start(wt[:], w0p[mh * 4 + c])
                    for kk in range(4):
                        k = c * 4 + kk
                        for mi in range(8):
                            nc.tensor.matmul(
                                ps[mi][:],
                                wt[:, kk * 1024 + mi * 128 : kk * 1024 + (mi + 1) * 128],
                                x0sb[:, k * NCOL : (k + 1) * NCOL],
                                start=(k == 0),
                                stop=(k == 15),
                            )
                for mi in range(8):
                    m = mh * 8 + mi
                    nc.scalar.activation(
                        x1sb[:, m * NCOL : (m + 1) * NCOL],
                        ps[mi][:],
                        Relu,
                        bias=b0sb[:, m : m + 1],
                    )

            # ---- stage 2: x2.T = relu(W1 @ x1.T + b1), M=1024
            ps2 = [
                psum.tile([P, NCOL], f32, tag="ps", name=f"ps{i}") for i in range(8)
            ]
            for c in range(4):
                wt = wpool.tile([P, 4096], f16, tag="w", name="wt")
                eng = nc.sync if c % 2 == 0 else nc.scalar
                eng.dma_start(wt[:], w1p[c])
                for kk in range(4):
                    k = c * 4 + kk
                    for mi in range(8):
                        nc.tensor.matmul(
                            ps2[mi][:],
                            wt[:, kk * 1024 + mi * 128 : kk * 1024 + (mi + 1) * 128],
                            x1sb[:, k * NCOL : (k + 1) * NCOL],
                            start=(k == 0),
                            stop=(k == 15),
                        )
            for mi in range(8):
                nc.scalar.activation(
                    x2sb[:, mi * NCOL : (mi + 1) * NCOL],
                    ps2[mi][:],
                    Relu,
                    bias=b1sb[:, mi : mi + 1],
                )

            # ---- stage 3: per group g: z_g.T = relu(L0[c_g] @ x2_g.T + O0)
            # l0p[g] holds L0[c_g].T as 8 k-tiles of [128, 512] side by side
            lts = []
            for h in range(4):
                lt = l0pool.tile([P, 4096], f16, tag="l0", name=f"lt{h}")
                eng = nc.sync if h % 2 == 0 else nc.scalar
                eng.dma_start(lt[:], l0p[h])
                lts.append(lt)
            for g in range(GROUPS_PER_CORE):
                ps3 = [
                    psum.tile([P, G], f32, tag="ps", name=f"ps3_{i}")
                    for i in range(4)
                ]
                wt = lts[g]
                base = 0
                for k in range(8):
                    for mi in range(4):
                        nc.tensor.matmul(
                            ps3[mi][:],
                            wt[:, base + k * 512 + mi * 128 : base + k * 512 + (mi + 1) * 128],
                            x2sb[:, k * NCOL + g * G : k * NCOL + (g + 1) * G],
                            start=(k == 0),
                            stop=(k == 7),
                        )
                for mi in range(4):
                    # fused (psum + O0) then max(0) on the otherwise-idle DVE
                    nc.vector.tensor_scalar(
                        zsb[:, (g * 4 + mi) * G : (g * 4 + mi + 1) * G],
                        ps3[mi][:],
                        o0sb[:, g * 4 + mi : g * 4 + mi + 1],
                        0.0,
                        Add,
                        Max,
                    )

            # ---- stage 4: y_g = L1[c_g] @ z_g.T + O1  -> [1, G] per group
            for g in range(GROUPS_PER_CORE):
                ps4 = psum.tile([1, G], f32, tag="ps", name="ps4")
                for k in range(4):
                    nc.tensor.matmul(
                        ps4[:],
                        l1sb[:, g * 4 + k : g * 4 + k + 1],
                        zsb[:, (g * 4 + k) * G : (g * 4 + k + 1) * G],
                        start=(k == 0),
                        stop=(k == 3),
                    )
                nc.scalar.activation(
                    ysb[0:1, g * G : (g + 1) * G],
                    ps4[0:1, :],
                    Identity,
                    bias=o1sb[0:1, g : g + 1],
                )

            nc.scalar.dma_start(y[:], ysb[:])

    nc.compile()
    return nc


def kernel(**inputs):
    global LAST_RUN
    import os

    from concourse.bass_utils import run_bass_kernel_spmd

    pairs = np.asarray(inputs["pairs"]).astype(np.int64)
    cell_lines = np.asarray(inputs["cell_lines"]).astype(np.int64)
    attrs = np.asarray(inputs["attrs"], dtype=np.float32)
    h_drug = np.asarray(inputs["h_drug"], dtype=np.float32)
    W0 = np.asarray(inputs["W0"], dtype=np.float32)
    b0 = np.asarray(inputs["b0"], dtype=np.float32)
    W1 = np.asarray(inputs["W1"], dtype=np.float32)
    b1 = np.asarray(inputs["b1"], dtype=np.float32)
    L0 = np.asarray(inputs["L0"], dtype=np.float32)
    O0 = np.asarray(inputs["O0"], dtype=np.float32)
    L1 = np.asarray(inputs["L1"], dtype=np.float32)
    O1 = np.asarray(inputs["O1"], dtype=np.float32)

    n_attr = attrs.shape[1] // 2
    # x0.T, feature-major: [2048, B]
    x0T = np.empty((D_IN, B), dtype=np.float32)
    x0T[:1023] = h_drug[pairs[:, 0]].T
    x0T[1023] = attrs[:, n_attr - 1]
    x0T[1024:2047] = h_drug[pairs[:, 1]].T
    x0T[2047] = attrs[:, -1]

    counts = np.bincount(cell_lines, minlength=N_CELL)
    G = max(8, int(-(-counts.max() // 8) * 8))
    NCOL = GROUPS_PER_CORE * G
    # one PSUM bank per [128, NCOL] f32 accumulator; 8 live at once
    assert NCOL <= 512, f"group padding {G} too large for single-bank PSUM tiles"
    groups = [np.where(cell_lines == c)[0] for c in range(N_CELL)]

    # shared (replicated) weight packs, bf16, chunk-of-4-ktiles layout
    w0k = W0.reshape(2, 1024, 16, P).transpose(0, 2, 3, 1).reshape(32, P, 1024)
    w0p = np.ascontiguousarray(
        w0k.reshape(2, 4, 4, P, 1024).transpose(0, 1, 3, 2, 4).reshape(8, P, 4096)
    ).astype(np.float16)
    w1k = W1.T.reshape(16, P, 1024)
    w1p = np.ascontiguousarray(
        w1k.reshape(4, 4, P, 1024).transpose(0, 2, 1, 3).reshape(4, P, 4096)
    ).astype(np.float16)
    b0m = np.ascontiguousarray(b0.reshape(16, P).T)
    b1m = np.ascontiguousarray(b1.reshape(8, P).T)

    in_maps = []
    for core in range(N_CORE):
        cells = [GROUPS_PER_CORE * core + i for i in range(GROUPS_PER_CORE)]
        x0c = np.zeros((D_IN, NCOL), dtype=np.float32)
        for gi, c in enumerate(cells):
            idx = groups[c]
            x0c[:, gi * G : gi * G + len(idx)] = x0T[:, idx]
        x0p = np.ascontiguousarray(
            x0c.reshape(16, P, NCOL).transpose(1, 0, 2).reshape(P, 16 * NCOL)
        ).astype(np.float16)
        # l0p[g] = L0[c_g].T as [8 ktiles, 128, 512] -> [128, 8*512]
        l0p = np.ascontiguousarray(
            np.stack(
                [
                    L0[c].T.reshape(8, P, 512).transpose(1, 0, 2).reshape(P, 4096)
                    for c in cells
                ]
            )
        ).astype(np.float16)
        o0m = np.ascontiguousarray(
            np.stack([O0[c][:, 0].reshape(4, P) for c in cells])
            .transpose(2, 0, 1)
            .reshape(P, 16)
        )
        l1m = np.ascontiguousarray(
            np.stack([L1[c][0].reshape(4, P) for c in cells])
            .transpose(2, 0, 1)
            .reshape(P, 16)
        ).astype(np.float16)
        o1m = np.ascontiguousarray(
            np.array([[O1[c, 0, 0] for c in cells]], dtype=np.float32)
        )
        in_maps.append(
            {
                "x0p": x0p,
                "w0p": w0p,
                "w1p": w1p,
                "l0p": l0p,
                "b0m": b0m,
                "b1m": b1m,
                "o0m": o0m,
                "l1m": l1m,
                "o1m": o1m,
            }
        )

    nc = _get_program(G)
    trace = bool(os.environ.get("BENCH_TRACE"))
    LAST_RUN = run_bass_kernel_spmd(nc, in_maps, list(range(N_CORE)), trace=trace)
    results = LAST_RUN.results

    out = np.zeros(B, dtype=np.float32)
    for core in range(N_CORE):
        yc = results[core]["y"]
        for gi in range(GROUPS_PER_CORE):
            c = GROUPS_PER_CORE * core + gi
            idx = groups[c]
            out[idx] = yc[0, gi * G : gi * G + len(idx)]
    return out
